# revision 5
# baseline (speedup 1.0000x reference)
"""GAT-style edge-softmax self-attention, dense-mask formulation, 8-core SPMD.

Math: per batch b (one NeuronCore per batch),
  Q/K/V = X @ Wq/k/v ; per head h: S = Q_h K_h^T / 8
  ex = C * exp(S)           (C[i,j] = multiplicity of edge (i<-j); softmax is
                             shift-invariant and |S| <~ 7, so no row-max needed)
  out_i = (ex @ V)_i / max(sum_j ex_ij, 1e-9)

v2 design notes (all timings per core):
  - scores: head PAIRS run concurrently via PE row tiling (K=64 each,
    tile_position (0,0)/(64,0)) into a 4-bank PSUM group.
  - exp: one ACT instruction per 4-bank group (free dim 2048) to amortize
    the 352-cycle ACTIVATE overhead.
  - mask multiply: all-bf16 tensor_tensor (DVE 2x mode), mask block is
    broadcast across the 2 heads of the group via a stride-0 AP dim.
  - AV: V is the stationary operand (65 cols = 64 V cols + ones column for
    the softmax denominator), exm streams with free dim 512.  Output lands
    as [feature, node] in PSUM; transposed back via PE transpose-mode,
    then normalized by the reciprocal denominator (stride-0 broadcast TT).
  - output tensor is bf16 on device; host upcasts to float32.
"""

import numpy as np
import ml_dtypes

import concourse.bass as bass
import concourse.bacc as bacc
import concourse.mybir as mybir
import concourse.tile as tile
from concourse.bass_utils import run_bass_kernel_spmd

B, N, H = 8, 1024, 768
NH, HD = 12, 64
P = 128
KC = H // P   # 6 contraction chunks for projections
JC = N // P   # 8 node chunks
NHP = NH // 2  # head pairs
F32 = mybir.dt.float32
BF16 = mybir.dt.bfloat16

_CACHE = {}


def _build_nc():
    nc = bacc.Bacc("TRN2", target_bir_lowering=False, debug=True)

    xT_d = nc.dram_tensor("xT", [H, N], BF16, kind="ExternalInput")
    wq_d = nc.dram_tensor("wq", [H, H], BF16, kind="ExternalInput")
    wk_d = nc.dram_tensor("wk", [H, H], BF16, kind="ExternalInput")
    wv_d = nc.dram_tensor("wv", [H, H], BF16, kind="ExternalInput")
    # mask, device layout: [p, i2*4096 + jc*512 + io] (j = jc*128+p, i = i2*512+io)
    mT_d = nc.dram_tensor("maskT", [P, JC * N], BF16, kind="ExternalInput")
    id_d = nc.dram_tensor("ident", [P, P], BF16, kind="ExternalInput")
    out_d = nc.dram_tensor("out", [N, H], BF16, kind="ExternalOutput")

    with tile.TileContext(nc) as tc:
        with tc.tile_pool(name="res", bufs=1) as res, \
             tc.tile_pool(name="work", bufs=2) as work:

            # ---- resident loads ----
            xT = [res.tile([P, N], BF16, tag=f"xT{k}", name=f"xT{k}") for k in range(KC)]
            wq = [res.tile([P, H], BF16, tag=f"wq{k}", name=f"wq{k}") for k in range(KC)]
            wk = [res.tile([P, H], BF16, tag=f"wk{k}", name=f"wk{k}") for k in range(KC)]
            wv = [res.tile([P, H], BF16, tag=f"wv{k}", name=f"wv{k}") for k in range(KC)]
            mT = res.tile([P, JC * N], BF16, tag="mT", name="mT")
            ident = res.tile([P, P], BF16, tag="ident", name="ident")
            nc.default_dma_engine.dma_start(out=ident[:], in_=id_d[:, :])
            for k in range(KC):
                nc.default_dma_engine.dma_start(out=xT[k][:], in_=xT_d[k * P:(k + 1) * P, :])
                nc.default_dma_engine.dma_start(out=wq[k][:], in_=wq_d[k * P:(k + 1) * P, :])
                nc.default_dma_engine.dma_start(out=wk[k][:], in_=wk_d[k * P:(k + 1) * P, :])
                nc.default_dma_engine.dma_start(out=wv[k][:], in_=wv_d[k * P:(k + 1) * P, :])
            for j in range(JC):
                nc.default_dma_engine.dma_start(
                    out=mT[:, j * N:(j + 1) * N], in_=mT_d[:, j * N:(j + 1) * N])

            # computed residents
            qT = [res.tile([P, N], BF16, tag=f"qT{k}", name=f"qT{k}") for k in range(KC)]
            kT = [res.tile([P, N], BF16, tag=f"kT{k}", name=f"kT{k}") for k in range(KC)]
            # V packed per head with a trailing ones column: cols h*65..h*65+63
            # hold V_h, col h*65+64 holds 1.0 (denominator trick).
            vp = [res.tile([P, NH * (HD + 1)], BF16, tag=f"vp{j}", name=f"vp{j}") for j in range(JC)]
            # final output, [p, ic*768 + h*64 + f], bf16
            outt = res.tile([P, JC * H], BF16, tag="outt", name="outt")

            # ---- projections ----
            with tc.tile_pool(name="pps", bufs=2, space="PSUM") as pps:
                for w_sb, dst in ((wq, qT), (wk, kT)):
                    for mo in range(KC):
                        for nn in range(2):
                            ps = pps.tile([P, 512], F32, tag="proj")
                            for k in range(KC):
                                nc.tensor.matmul(
                                    ps[:],
                                    w_sb[k][:, mo * P:(mo + 1) * P],
                                    xT[k][:, nn * 512:(nn + 1) * 512],
                                    start=(k == 0), stop=(k == KC - 1),
                                )
                            nc.any.tensor_copy(
                                out=dst[mo][:, nn * 512:(nn + 1) * 512], in_=ps[:])
                # V: out (j nodes, feat) = X @ Wv ; pack into vp with ones cols
                for j in range(JC):
                    nc.gpsimd.memset(vp[j][:], 1.0)
                    for nn, (c0, cw, nh) in enumerate(((0, 512, 8), (512, 256, 4))):
                        ps = pps.tile([P, 512], F32, tag="proj")
                        for k in range(KC):
                            nc.tensor.matmul(
                                ps[:, :cw],
                                xT[k][:, j * P:(j + 1) * P],
                                wv[k][:, c0:c0 + cw],
                                start=(k == 0), stop=(k == KC - 1),
                            )
                        h0 = c0 // HD
                        src = ps[:, 0:cw].rearrange("p (h x) -> p h x", h=nh)
                        dst = vp[j][:, h0 * (HD + 1):(h0 + nh) * (HD + 1)] \
                            .rearrange("p (h x) -> p h x", h=nh)[:, :, 0:HD]
                        nc.any.tensor_copy(out=dst, in_=src)

            # ---- main attention loop ----
            with tc.tile_pool(name="sps", bufs=1, space="PSUM") as spsp, \
                 tc.tile_pool(name="ops", bufs=1, space="PSUM") as opsp, \
                 tc.tile_pool(name="p2p", bufs=2, space="PSUM") as p2p:
                for hp in range(NHP):
                    hA, hB = 2 * hp, 2 * hp + 1
                    kt, qt = kT[hp], qT[hp]
                    for i2 in range(2):
                        oAB = [opsp.tile([P, 512], F32, tag=f"o{x}", name=f"o{x}_{hp}_{i2}")
                               for x in "AB"]
                        for jg in range(4):
                            S4 = spsp.tile([P, 2048], F32, tag="S4")
                            for jo in range(2):
                                j = jg * 2 + jo
                                nc.tensor.matmul(
                                    S4[:, (2 * jo) * 512:(2 * jo + 1) * 512],
                                    kt[0:HD, j * P:(j + 1) * P],
                                    qt[0:HD, i2 * 512:(i2 + 1) * 512],
                                    start=True, stop=True, tile_position=(0, 0))
                                nc.tensor.matmul(
                                    S4[:, (2 * jo + 1) * 512:(2 * jo + 2) * 512],
                                    kt[HD:P, j * P:(j + 1) * P],
                                    qt[HD:P, i2 * 512:(i2 + 1) * 512],
                                    start=True, stop=True, tile_position=(64, 0))
                            EX = work.tile([P, 2048], BF16, tag="EX")
                            nc.scalar.activation(
                                EX[:], S4[:],
                                mybir.ActivationFunctionType.Exp, scale=0.125)
                            XM = work.tile([P, 2048], BF16, tag="XM")
                            base = i2 * 4096 + jg * 1024
                            m_ap = mT[:, base:base + 1024] \
                                .rearrange("p (j x) -> p j x", j=2) \
                                .unsqueeze(2).broadcast_to((P, 2, 2, 512))
                            nc.vector.tensor_tensor(
                                out=XM[:].rearrange("p (j h x) -> p j h x", j=2, h=2),
                                in0=EX[:].rearrange("p (j h x) -> p j h x", j=2, h=2),
                                in1=m_ap, op=mybir.AluOpType.mult)
                            for jo in range(2):
                                j = jg * 2 + jo
                                first = (jg == 0 and jo == 0)
                                last = (jg == 3 and jo == 1)
                                nc.tensor.matmul(
                                    oAB[0][0:HD + 1, :],
                                    vp[j][:, hA * (HD + 1):(hA + 1) * (HD + 1)],
                                    XM[:, (2 * jo) * 512:(2 * jo + 1) * 512],
                                    start=first, stop=last)
                                nc.tensor.matmul(
                                    oAB[1][0:HD + 1, :],
                                    vp[j][:, hB * (HD + 1):(hB + 1) * (HD + 1)],
                                    XM[:, (2 * jo + 1) * 512:(2 * jo + 2) * 512],
                                    start=first, stop=last)
                        # output: transpose + normalize per head
                        for h, o in ((hA, oAB[0]), (hB, oAB[1])):
                            oraw = work.tile([P, 512], BF16, tag="oraw")
                            nc.any.tensor_copy(out=oraw[0:HD + 1, :], in_=o[0:HD + 1, :])
                            # per-s block stride 66 cols (132B) keeps bf16 PSUM
                            # writes 4-byte aligned
                            SW = HD + 2
                            P2 = p2p.tile([P, 4 * SW], BF16, tag="P2")
                            for s in range(4):
                                nc.tensor.transpose(
                                    P2[:, s * SW:s * SW + HD + 1],
                                    oraw[0:HD + 1, s * P:(s + 1) * P],
                                    ident[0:HD + 1, 0:HD + 1])
                            rec = work.tile([P, 4], F32, tag="rec")
                            for s in range(4):
                                nc.vector.tensor_scalar_max(
                                    rec[:, s:s + 1],
                                    P2[:, s * SW + HD:s * SW + HD + 1],
                                    1e-9)
                            nc.vector.reciprocal(rec[:], rec[:])
                            src = P2[:].rearrange("p (s x) -> p s x", s=4)[:, :, 0:HD]
                            r_b = rec[:].unsqueeze(2).broadcast_to((P, 4, HD))
                            dst = outt[:, i2 * 4 * H:(i2 + 1) * 4 * H] \
                                .rearrange("p (s x) -> p s x", s=4)[:, :, h * HD:(h + 1) * HD]
                            nc.vector.tensor_tensor(
                                out=dst, in0=src, in1=r_b, op=mybir.AluOpType.mult)

            for ic in range(JC):
                nc.default_dma_engine.dma_start(
                    out=out_d[ic * P:(ic + 1) * P, :], in_=outt[:, ic * H:(ic + 1) * H])

    nc.compile()
    return nc


def _prep_in_maps(node_states, edge_indices, Wq, Wk, Wv):
    eb, ei, ej = (np.asarray(edge_indices[r]) for r in range(3))
    idx = (eb.astype(np.int64) * N + ej) * N + ei
    CT = np.bincount(idx, minlength=B * N * N).astype(np.float32).reshape(B, N, N)
    # device mask layout: [p, i2*4096 + jc*512 + io]
    CTd = CT.reshape(B, JC, P, 2, 512).transpose(0, 2, 3, 1, 4).reshape(B, P, JC * N)

    bf = ml_dtypes.bfloat16
    wq = np.ascontiguousarray(Wq).astype(bf)
    wk = np.ascontiguousarray(Wk).astype(bf)
    wv = np.ascontiguousarray(Wv).astype(bf)
    ident = np.eye(P, dtype=bf)

    in_maps = []
    for b in range(B):
        in_maps.append({
            "xT": np.ascontiguousarray(np.asarray(node_states[b]).T).astype(bf),
            "wq": wq, "wk": wk, "wv": wv,
            "maskT": np.ascontiguousarray(CTd[b]).astype(bf),
            "ident": ident,
        })
    return in_maps


def kernel(node_states, edge_indices, Wq, Wk, Wv):
    if "nc" not in _CACHE:
        _CACHE["nc"] = _build_nc()
    nc = _CACHE["nc"]
    in_maps = _prep_in_maps(node_states, edge_indices, Wq, Wk, Wv)
    res = run_bass_kernel_spmd(nc, in_maps, list(range(B)))
    out = np.stack([np.asarray(res.results[b]["out"]) for b in range(B)], axis=0)
    return out.astype(np.float32)


def run_traced(inputs, **kw):
    if "nc" not in _CACHE:
        _CACHE["nc"] = _build_nc()
    nc = _CACHE["nc"]
    in_maps = _prep_in_maps(**inputs)
    return run_bass_kernel_spmd(nc, in_maps, list(range(B)), trace=True, **kw)


# revision 9
# speedup vs baseline: 1.2885x; 1.2885x over previous
"""GAT-style edge-softmax self-attention, dense-mask formulation, 8-core SPMD.

Math: per batch b (one NeuronCore per batch),
  Q/K/V = X @ Wq/k/v ; per head h: S = Q_h K_h^T / 8
  ex = C * exp(S)           (C[i,j] = multiplicity of edge (i<-j); softmax is
                             shift-invariant and |S| <~ 7, so no row-max needed)
  out_i = (ex @ V)_i / max(sum_j ex_ij, 1e-9)

v2 design notes (all timings per core):
  - scores: head PAIRS run concurrently via PE row tiling (K=64 each,
    tile_position (0,0)/(64,0)) into a 4-bank PSUM group.
  - exp: one ACT instruction per 4-bank group (free dim 2048) to amortize
    the 352-cycle ACTIVATE overhead.
  - mask multiply: all-bf16 tensor_tensor (DVE 2x mode), mask block is
    broadcast across the 2 heads of the group via a stride-0 AP dim.
  - AV: V is the stationary operand (65 cols = 64 V cols + ones column for
    the softmax denominator), exm streams with free dim 512.  Output lands
    as [feature, node] in PSUM; transposed back via PE transpose-mode,
    then normalized by the reciprocal denominator (stride-0 broadcast TT).
  - output tensor is bf16 on device; host upcasts to float32.
"""

import numpy as np
import ml_dtypes

import concourse.bass as bass
import concourse.bacc as bacc
import concourse.mybir as mybir
import concourse.tile as tile
from concourse.bass_utils import run_bass_kernel_spmd

B, N, H = 8, 1024, 768
NH, HD = 12, 64
P = 128
KC = H // P   # 6 contraction chunks for projections
JC = N // P   # 8 node chunks
NHP = NH // 2  # head pairs
F32 = mybir.dt.float32
BF16 = mybir.dt.bfloat16

_CACHE = {}


def _build_nc():
    nc = bacc.Bacc("TRN2", target_bir_lowering=False, debug=True)

    xT_d = nc.dram_tensor("xT", [H, N], BF16, kind="ExternalInput")
    wq_d = nc.dram_tensor("wq", [H, H], BF16, kind="ExternalInput")
    wk_d = nc.dram_tensor("wk", [H, H], BF16, kind="ExternalInput")
    wv_d = nc.dram_tensor("wv", [H, H], BF16, kind="ExternalInput")
    # mask, device layout: [p, i2*4096 + jc*512 + io] (j = jc*128+p, i = i2*512+io)
    mT_d = nc.dram_tensor("maskT", [P, JC * N], BF16, kind="ExternalInput")
    id_d = nc.dram_tensor("ident", [P, P], BF16, kind="ExternalInput")
    out_d = nc.dram_tensor("out", [N, H], BF16, kind="ExternalOutput")

    with tile.TileContext(nc) as tc:
        with tc.tile_pool(name="res", bufs=1) as res, \
             tc.tile_pool(name="work", bufs=2) as work:

            # ---- resident loads ----
            xT = [res.tile([P, N], BF16, tag=f"xT{k}", name=f"xT{k}") for k in range(KC)]
            wq = [res.tile([P, H], BF16, tag=f"wq{k}", name=f"wq{k}") for k in range(KC)]
            wk = [res.tile([P, H], BF16, tag=f"wk{k}", name=f"wk{k}") for k in range(KC)]
            wv = [res.tile([P, H], BF16, tag=f"wv{k}", name=f"wv{k}") for k in range(KC)]
            mT = res.tile([P, JC * N], BF16, tag="mT", name="mT")
            ident = res.tile([P, P], BF16, tag="ident", name="ident")
            nc.default_dma_engine.dma_start(out=ident[:], in_=id_d[:, :])
            for k in range(KC):
                nc.default_dma_engine.dma_start(out=xT[k][:], in_=xT_d[k * P:(k + 1) * P, :])
                nc.default_dma_engine.dma_start(out=wq[k][:], in_=wq_d[k * P:(k + 1) * P, :])
                nc.default_dma_engine.dma_start(out=wk[k][:], in_=wk_d[k * P:(k + 1) * P, :])
                nc.default_dma_engine.dma_start(out=wv[k][:], in_=wv_d[k * P:(k + 1) * P, :])
            for j in range(JC):
                nc.default_dma_engine.dma_start(
                    out=mT[:, j * N:(j + 1) * N], in_=mT_d[:, j * N:(j + 1) * N])

            # computed residents
            qT = [res.tile([P, N], BF16, tag=f"qT{k}", name=f"qT{k}") for k in range(KC)]
            kT = [res.tile([P, N], BF16, tag=f"kT{k}", name=f"kT{k}") for k in range(KC)]
            # V packed per head with a trailing ones column: cols h*65..h*65+63
            # hold V_h, col h*65+64 holds 1.0 (denominator trick).
            vp = [res.tile([P, NH * (HD + 1)], BF16, tag=f"vp{j}", name=f"vp{j}") for j in range(JC)]
            # final output, [p, ic*768 + h*64 + f], bf16
            outt = res.tile([P, JC * H], BF16, tag="outt", name="outt")

            # ---- projections ----
            with tc.tile_pool(name="pps", bufs=2, space="PSUM") as pps:
                for w_sb, dst in ((wq, qT), (wk, kT)):
                    for mo in range(KC):
                        for nn in range(2):
                            ps = pps.tile([P, 512], F32, tag="proj")
                            for k in range(KC):
                                nc.tensor.matmul(
                                    ps[:],
                                    w_sb[k][:, mo * P:(mo + 1) * P],
                                    xT[k][:, nn * 512:(nn + 1) * 512],
                                    start=(k == 0), stop=(k == KC - 1),
                                )
                            nc.scalar.activation(
                                dst[mo][:, nn * 512:(nn + 1) * 512], ps[:],
                                mybir.ActivationFunctionType.Copy)
                # V: out (j nodes, feat) = X @ Wv ; pack into vp with ones cols
                for j in range(JC):
                    nc.gpsimd.memset(vp[j][:], 1.0)
                    for nn, (c0, cw, nh) in enumerate(((0, 512, 8), (512, 256, 4))):
                        ps = pps.tile([P, 512], F32, tag="proj")
                        for k in range(KC):
                            nc.tensor.matmul(
                                ps[:, :cw],
                                xT[k][:, j * P:(j + 1) * P],
                                wv[k][:, c0:c0 + cw],
                                start=(k == 0), stop=(k == KC - 1),
                            )
                        h0 = c0 // HD
                        src = ps[:, 0:cw].rearrange("p (h x) -> p h x", h=nh)
                        dst = vp[j][:, h0 * (HD + 1):(h0 + nh) * (HD + 1)] \
                            .rearrange("p (h x) -> p h x", h=nh)[:, :, 0:HD]
                        nc.scalar.activation(
                            dst, src, mybir.ActivationFunctionType.Copy)

            # ---- main attention loop ----
            with tc.tile_pool(name="sps", bufs=2, space="PSUM") as spsp, \
                 tc.tile_pool(name="ops", bufs=1, space="PSUM") as opsp, \
                 tc.tile_pool(name="p2p", bufs=2, space="PSUM") as p2p:
                for hp in range(NHP):
                    hA, hB = 2 * hp, 2 * hp + 1
                    kt, qt = kT[hp], qT[hp]
                    for i2 in range(2):
                        oAB = [opsp.tile([P, 512], F32, tag=f"o{x}", name=f"o{x}_{hp}_{i2}")
                               for x in "AB"]
                        for j in range(JC):
                            # 2-bank group: [head A | head B] scores for node
                            # chunk j; double-buffered so scores of j+1 overlap
                            # exp/mult/AV of j.
                            S2 = spsp.tile([P, 1024], F32, tag="S2")
                            nc.tensor.matmul(
                                S2[:, 0:512],
                                kt[0:HD, j * P:(j + 1) * P],
                                qt[0:HD, i2 * 512:(i2 + 1) * 512],
                                start=True, stop=True, tile_position=(0, 0))
                            nc.tensor.matmul(
                                S2[:, 512:1024],
                                kt[HD:P, j * P:(j + 1) * P],
                                qt[HD:P, i2 * 512:(i2 + 1) * 512],
                                start=True, stop=True, tile_position=(64, 0))
                            EX = work.tile([P, 1024], BF16, tag="EX")
                            nc.scalar.activation(
                                EX[:], S2[:],
                                mybir.ActivationFunctionType.Exp, scale=0.125)
                            XM = work.tile([P, 1024], BF16, tag="XM")
                            base = i2 * 4096 + j * 512
                            m_ap = mT[:, base:base + 512] \
                                .unsqueeze(1).broadcast_to((P, 2, 512))
                            nc.vector.tensor_tensor(
                                out=XM[:].rearrange("p (h x) -> p h x", h=2),
                                in0=EX[:].rearrange("p (h x) -> p h x", h=2),
                                in1=m_ap, op=mybir.AluOpType.mult)
                            first, last = (j == 0), (j == JC - 1)
                            nc.tensor.matmul(
                                oAB[0][0:HD + 1, :],
                                vp[j][:, hA * (HD + 1):(hA + 1) * (HD + 1)],
                                XM[:, 0:512], start=first, stop=last)
                            nc.tensor.matmul(
                                oAB[1][0:HD + 1, :],
                                vp[j][:, hB * (HD + 1):(hB + 1) * (HD + 1)],
                                XM[:, 512:1024], start=first, stop=last)
                        # output: transpose + normalize per head
                        for h, o in ((hA, oAB[0]), (hB, oAB[1])):
                            oraw = work.tile([P, 512], BF16, tag="oraw")
                            nc.vector.tensor_copy(out=oraw[0:HD + 1, :], in_=o[0:HD + 1, :])
                            # per-s block stride 66 cols (132B) keeps bf16 PSUM
                            # writes 4-byte aligned
                            SW = HD + 2
                            P2 = p2p.tile([P, 4 * SW], BF16, tag="P2")
                            for s in range(4):
                                nc.tensor.transpose(
                                    P2[:, s * SW:s * SW + HD + 1],
                                    oraw[0:HD + 1, s * P:(s + 1) * P],
                                    ident[0:HD + 1, 0:HD + 1])
                            rec = work.tile([P, 4], F32, tag="rec")
                            den_ap = P2[:].rearrange("p (s x) -> p s x", s=4)[:, :, HD:HD + 1]
                            nc.vector.tensor_scalar_max(rec[:].unsqueeze(2), den_ap, 1e-9)
                            nc.vector.reciprocal(rec[:], rec[:])
                            src = P2[:].rearrange("p (s x) -> p s x", s=4)[:, :, 0:HD]
                            r_b = rec[:].unsqueeze(2).broadcast_to((P, 4, HD))
                            dst = outt[:, i2 * 4 * H:(i2 + 1) * 4 * H] \
                                .rearrange("p (s x) -> p s x", s=4)[:, :, h * HD:(h + 1) * HD]
                            nc.vector.tensor_tensor(
                                out=dst, in0=src, in1=r_b, op=mybir.AluOpType.mult)

            for ic in range(JC):
                nc.default_dma_engine.dma_start(
                    out=out_d[ic * P:(ic + 1) * P, :], in_=outt[:, ic * H:(ic + 1) * H])

    nc.compile()
    return nc


def _prep_in_maps(node_states, edge_indices, Wq, Wk, Wv):
    eb, ei, ej = (np.asarray(edge_indices[r]) for r in range(3))
    idx = (eb.astype(np.int64) * N + ej) * N + ei
    CT = np.bincount(idx, minlength=B * N * N).astype(np.float32).reshape(B, N, N)
    # device mask layout: [p, i2*4096 + jc*512 + io]
    CTd = CT.reshape(B, JC, P, 2, 512).transpose(0, 2, 3, 1, 4).reshape(B, P, JC * N)

    bf = ml_dtypes.bfloat16
    wq = np.ascontiguousarray(Wq).astype(bf)
    wk = np.ascontiguousarray(Wk).astype(bf)
    wv = np.ascontiguousarray(Wv).astype(bf)
    ident = np.eye(P, dtype=bf)

    in_maps = []
    for b in range(B):
        in_maps.append({
            "xT": np.ascontiguousarray(np.asarray(node_states[b]).T).astype(bf),
            "wq": wq, "wk": wk, "wv": wv,
            "maskT": np.ascontiguousarray(CTd[b]).astype(bf),
            "ident": ident,
        })
    return in_maps


def kernel(node_states, edge_indices, Wq, Wk, Wv):
    if "nc" not in _CACHE:
        _CACHE["nc"] = _build_nc()
    nc = _CACHE["nc"]
    in_maps = _prep_in_maps(node_states, edge_indices, Wq, Wk, Wv)
    res = run_bass_kernel_spmd(nc, in_maps, list(range(B)))
    out = np.stack([np.asarray(res.results[b]["out"]) for b in range(B)], axis=0)
    return out.astype(np.float32)


def run_traced(inputs, **kw):
    if "nc" not in _CACHE:
        _CACHE["nc"] = _build_nc()
    nc = _CACHE["nc"]
    in_maps = _prep_in_maps(**inputs)
    return run_bass_kernel_spmd(nc, in_maps, list(range(B)), trace=True, **kw)
